# revision 42
# baseline (speedup 1.0000x reference)
"""Banded (Luong) attention TRN2 Bass kernel, 8-core SPMD.

Problem: h [4, 4096, 1024] f32, W [1024, 1024] f32, T_hist=256.
  K = h @ W.T ; scores = (h @ K^T) / sqrt(H) with causal band
  (q - 255 <= k <= q); out = softmax(scores) @ h.

Sharding: data-parallel over batch (4) x sequence halves (2) -> 8 cores,
no cross-core communication. Each core handles 2048 queries; its key
region is 18 blocks of 128 (2 lead blocks for the band history,
zero-padded for the first half of each sequence).

Per-core algorithm, all matmul operands in bf16 (host-cast; halves HBM
traffic), fp32 PSUM accumulate:
  warm-up matmuls on a memset tile absorb the HAM cold window while
      the first DMAs land; input triggers are spread across the sync/
      scalar/vector/gpsimd queues so descriptor generation overlaps
  Q'T[m, q] = W^T @ hT          full 2048-query projection first
  per key block r (18): ST_r[k, q] = h_r @ Q'T over the <=384 queries
      whose band covers r; masked additively, exp with fused 1/32 scale
  per query block i (ctx delayed 3 score iterations): ctx = PT^T @ h
      over 3 key blocks; row sums via a ones-column matmul; normalized
      on the vector engine into bf16 and DMA'd per block from gpsimd.
"""

import numpy as np

import concourse.mybir as mybir
import concourse.tile as tile
from concourse import bacc

B, T, H = 4, 4096, 1024
T_HIST = 256
N_CORES = 8
QB = 16            # 128-row query blocks per core
RB = QB + 2        # key-region blocks per core (2 lead blocks)
QPC = 2048         # queries per core
BF16 = mybir.dt.bfloat16
F32 = mybir.dt.float32
NEG = np.float32(-1e9)
INV_SQRT_H = 1.0 / 32.0

_CACHE = {}


def _mask_slice(r, m_sb, bm_sb):
    # interior mask layout: [caus | full | su] for the ascending query
    # sub-blocks (j = 2, 1, 0) of key block r's band window
    if r == 0:
        return bm_sb[:, 256:384]
    if r == 1:
        return bm_sb[:, 128:384]
    if r == QB:
        return m_sb[:, 0:256]
    if r == QB + 1:
        return m_sb[:, 0:128]
    return m_sb[:, 0:384]


def _kernel_body(tc, out, hTr, haug, Wr, maskd, bmaskd, onesd):
    nc = tc.nc

    with (
        tc.tile_pool(name="singles", bufs=1) as singles,
        tc.tile_pool(name="pt", bufs=5) as pt_pool,
        tc.tile_pool(name="ctxs", bufs=3) as ctxs_pool,
        tc.tile_pool(name="recip", bufs=4) as recip_pool,
        tc.tile_pool(name="qtps", bufs=2, space="PSUM") as qtps_pool,
        tc.tile_pool(name="st", bufs=2, space="PSUM") as st_pool,
        tc.tile_pool(name="ctx", bufs=2, space="PSUM") as ctx_pool,
    ):
        # --- PE warm-up: absorb the HAM cold window while DMAs land ---
        scratch = singles.tile([128, 128], BF16)
        nc.gpsimd.memset(scratch[:], 0.0)
        wps = st_pool.tile([128, 384], F32, tag="st")

        def filler(n):
            for _ in range(n):
                nc.tensor.matmul(
                    wps[:, 0:128], scratch[:], scratch[:],
                    start=True, stop=True, skip_group_check=True,
                )

        filler(72)

        # --- resident inputs; triggers spread across idle queues so
        # descriptor generation overlaps; the first projection group's
        # operands (W slab 0, hT oc-chunks of the first query window)
        # are small so the PE starts within ~2us of the body barrier ---
        W_sb = singles.tile([128, 8, H], BF16)           # 2 MiB
        hT_sb = singles.tile([128, 8, RB * 128], BF16)   # 4.7 MiB
        for oc in range(0, 8, 2):
            nc.gpsimd.dma_start(
                hT_sb[:, oc, 256:768], hTr[:, oc, 256:768]
            )
            nc.scalar.dma_start(
                hT_sb[:, oc + 1, 256:768], hTr[:, oc + 1, 256:768]
            )
        nc.sync.dma_start(W_sb[:, 0:2, :], Wr[:, 0:2, :])
        nc.sync.dma_start(W_sb[:, 2:4, :], Wr[:, 2:4, :])
        nc.scalar.dma_start(W_sb[:, 4:8, :], Wr[:, 4:8, :])
        # later-phase inputs: scheduler wait hints keep these triggers
        # out of the critical first-window bandwidth
        with tc.tile_wait_until(0.015):
            nc.scalar.dma_start(hT_sb[:, :, 768:1280], hTr[:, :, 768:1280])
        m_sb = singles.tile([128, 384], F32)
        bm_sb = singles.tile([128, 384], F32)
        ones_sb = singles.tile([128, 2], BF16)
        ha_sb = singles.tile([128, RB, H], BF16)         # 4.7 MiB
        with tc.tile_wait_until(0.025):
            nc.sync.dma_start(hT_sb[:, :, 1280:2304], hTr[:, :, 1280:2304])
        with tc.tile_wait_until(0.03):
            nc.sync.dma_start(hT_sb[:, :, 0:256], hTr[:, :, 0:256])
            nc.sync.dma_start(m_sb[:], maskd[:])
            nc.sync.dma_start(bm_sb[:], bmaskd[:])
            nc.sync.dma_start(ones_sb[:], onesd[:])
            for hc in range(3):
                nc.sync.dma_start(
                    ha_sb[:, hc * 6 : (hc + 1) * 6, :],
                    haug[:, hc * 6 : (hc + 1) * 6, :],
                )

        # --- projection: Q'T[m, q] for all 2048 queries. The first
        # window runs oc-outer across 4 concurrent PSUM groups so the
        # PE stays dense while the hT oc-chunks trickle in from HBM ---
        qt = singles.tile([128, 8, QPC], BF16)           # 4 MiB
        ps0 = qtps_pool.tile([128, 512], F32, tag="qtps")
        ps1 = qtps_pool.tile([128, 512], F32, tag="qtps")
        ctx0 = ctx_pool.tile([128, H], F32, tag="ctx")
        groups = [ps0[:], ps1[:], ctx0[:, 0:512], ctx0[:, 512:1024]]
        for oc in range(8):
            for mc in range(4):
                nc.tensor.matmul(
                    groups[mc],
                    W_sb[:, mc, oc * 128 : (oc + 1) * 128],
                    hT_sb[:, oc, 256:768],
                    start=(oc == 0),
                    stop=(oc == 7),
                )
            if oc < 7:
                filler(3)   # absorb oc-chunk arrival jitter
        for mc in range(4):
            nc.vector.tensor_copy(qt[:, mc, 0:512], groups[mc])
        for tt in range(4):
            for mc in range(4 if tt == 0 else 0, 8):
                ps = qtps_pool.tile([128, 512], F32, tag="qtps")
                for oc in range(8):
                    nc.tensor.matmul(
                        ps[:],
                        W_sb[:, mc, oc * 128 : (oc + 1) * 128],
                        hT_sb[:, oc, 256 + tt * 512 : 256 + (tt + 1) * 512],
                        start=(oc == 0),
                        stop=(oc == 7),
                    )
                nc.vector.tensor_copy(
                    qt[:, mc, tt * 512 : (tt + 1) * 512], ps[:]
                )

        # --- banded attention: scores per key block, ctx per query
        # block lagging 3 iterations so softmax latency stays hidden ---
        pt_tiles = {}

        def scores(r):
            qb0 = max(0, r - 2)
            nq = (min(QB - 1, r) - qb0 + 1) * 128
            st = st_pool.tile([128, 384], F32, tag="st")
            for mc in range(8):
                nc.tensor.matmul(
                    st[:, 0:nq],
                    hT_sb[:, mc, r * 128 : (r + 1) * 128],
                    qt[:, mc, qb0 * 128 : qb0 * 128 + nq],
                    start=(mc == 0),
                    stop=(mc == 7),
                )
            nc.vector.tensor_add(
                st[:, 0:nq], st[:, 0:nq], _mask_slice(r, m_sb, bm_sb)
            )
            pt = pt_pool.tile([128, 384], BF16, tag="pt")
            nc.scalar.activation(
                pt[:, 0:nq], st[:, 0:nq], mybir.ActivationFunctionType.Exp,
                scale=INV_SQRT_H,
            )
            pt_tiles[r] = pt

        def context_mms(i, ctx, sums, js):
            for j in js:
                rr = i + j
                qoff = (i - max(0, rr - 2)) * 128
                lhsT = pt_tiles[rr][:, qoff : qoff + 128]
                nc.tensor.matmul(
                    ctx[:, 0:512], lhsT, ha_sb[:, rr, 0:512],
                    start=(j == 0), stop=(j == 2),
                )
                nc.tensor.matmul(
                    ctx[:, 512:1024], lhsT, ha_sb[:, rr, 512:1024],
                    start=(j == 0), stop=(j == 2),
                )
                nc.tensor.matmul(
                    sums[:], lhsT, ones_sb[:],
                    start=(j == 0), stop=(j == 2),
                )

        def context(i):
            ctx = ctx_pool.tile([128, H], F32, tag="ctx")
            sums = qtps_pool.tile([128, 2], F32, tag="qtps")
            context_mms(i, ctx, sums, (0, 1, 2))
            recip = recip_pool.tile([128, 1], F32)
            nc.vector.reciprocal(recip[:], sums[:, 0:1])
            ctxs = ctxs_pool.tile([128, H], BF16, tag="ctxs")
            if i % 2 == 0:
                nc.vector.tensor_scalar_mul(ctxs[:], ctx[:], recip[:])
            else:
                nc.scalar.mul(ctxs[:], ctx[:], mul=recip[:])
            nc.gpsimd.dma_start(out[i], ctxs[:])

        # ctx_15's pt_15/pt_16 contributions run before the last scores
        # iteration; only the pt_17 part remains on the critical tail
        ctx15 = sums15 = None
        for r in range(RB):
            scores(r)
            if r == RB - 2:
                context(r - 3)
                ctx15 = ctx_pool.tile([128, H], F32, tag="ctx")
                sums15 = qtps_pool.tile([128, 2], F32, tag="qtps")
                context_mms(QB - 1, ctx15, sums15, (0, 1))
            elif r == RB - 1:
                # finish ctx_15's matmuls first so its output chain
                # overlaps ctx_14's matmuls; ctx_14's DMA triggers go
                # on the idle sync queue, parallel with gpsimd's
                ctx14 = ctx_pool.tile([128, H], F32, tag="ctx")
                sums14 = qtps_pool.tile([128, 2], F32, tag="qtps")
                context_mms(QB - 2, ctx14, sums14, (0,))
                context_mms(QB - 1, ctx15, sums15, (2,))
                context_mms(QB - 2, ctx14, sums14, (1, 2))
                recip15 = recip_pool.tile([128, 1], F32)
                nc.vector.reciprocal(recip15[:], sums15[:, 0:1])
                ctxs15 = ctxs_pool.tile([128, H], BF16, tag="ctxs")
                nc.vector.tensor_scalar_mul(
                    ctxs15[:, 0:512], ctx15[:, 0:512], recip15[:]
                )
                nc.scalar.mul(
                    ctxs15[:, 512:1024], ctx15[:, 512:1024], mul=recip15[:]
                )
                nc.gpsimd.dma_start(out[QB - 1, :, 0:512], ctxs15[:, 0:512])
                nc.gpsimd.dma_start(
                    out[QB - 1, :, 512:1024], ctxs15[:, 512:1024]
                )
                recip14 = recip_pool.tile([128, 1], F32)
                nc.vector.reciprocal(recip14[:], sums14[:, 0:1])
                ctxs14 = ctxs_pool.tile([128, H], BF16, tag="ctxs")
                nc.scalar.mul(
                    ctxs14[:, 0:512], ctx14[:, 0:512], mul=recip14[:]
                )
                nc.vector.tensor_scalar_mul(
                    ctxs14[:, 512:1024], ctx14[:, 512:1024], recip14[:]
                )
                nc.sync.dma_start(out[QB - 2, :, 0:512], ctxs14[:, 0:512])
                nc.sync.dma_start(
                    out[QB - 2, :, 512:1024], ctxs14[:, 512:1024]
                )
            elif r >= 3:
                context(r - 3)


def _build():
    if "nc" in _CACHE:
        return _CACHE["nc"]
    nc = bacc.Bacc(
        "TRN2", target_bir_lowering=False, debug=False, num_devices=N_CORES
    )
    hTr = nc.dram_tensor("hTr", [128, 8, RB * 128], BF16, kind="ExternalInput").ap()
    haug = nc.dram_tensor("haug", [128, RB, H], BF16, kind="ExternalInput").ap()
    Wr = nc.dram_tensor("Wr", [128, 8, H], BF16, kind="ExternalInput").ap()
    maskd = nc.dram_tensor("maskd", [128, 384], F32, kind="ExternalInput").ap()
    bmaskd = nc.dram_tensor("bmaskd", [128, 384], F32, kind="ExternalInput").ap()
    onesd = nc.dram_tensor("onesd", [128, 2], BF16, kind="ExternalInput").ap()
    out = nc.dram_tensor("out", [QB, 128, H], BF16, kind="ExternalOutput").ap()
    with tile.TileContext(nc) as tc:
        _kernel_body(tc, out, hTr, haug, Wr, maskd, bmaskd, onesd)
    nc.compile()
    _CACHE["nc"] = nc
    return nc


def _host_masks():
    kk = np.arange(128, dtype=np.int64)[:, None]
    qi = np.arange(128, dtype=np.int64)[None, :]
    su = np.where(kk > qi, np.float32(0.0), NEG).astype(np.float32)
    caus = np.where(kk <= qi, np.float32(0.0), NEG).astype(np.float32)
    mask = np.empty((128, 384), np.float32)
    mask[:, 0:128] = caus
    mask[:, 128:256] = 0.0
    mask[:, 256:384] = su
    return mask


def _prepare_in_maps(h, W):
    import ml_dtypes

    bf16 = ml_dtypes.bfloat16
    mask = _host_masks()
    bmask_pad = np.full((128, 384), NEG, np.float32)
    # Wr[p, mc, oc*128 + j] = W[oc*128 + p, mc*128 + j]: per-partition
    # contiguous 2 KiB slabs per m-chunk for efficient DMA
    Wr = np.ascontiguousarray(
        W.reshape(8, 128, 8, 128).transpose(1, 2, 0, 3).reshape(128, 8, H)
    ).astype(bf16)
    ones = np.ones((128, 2), bf16)
    in_maps = []
    for core in range(N_CORES):
        b, half = core // 2, core % 2
        k_lo = half * QPC - 256            # region global key start
        pad = max(0, -k_lo)                # 256 for half 0, else 0
        k_lo = max(0, k_lo)
        k_hi = half * QPC + QPC

        hreg = np.zeros((RB * 128, H), np.float32)
        hreg[pad:] = h[b, k_lo:k_hi]
        hreg = hreg.astype(bf16)

        # feature-major region [128, 8, 2304]
        hTr = np.ascontiguousarray(
            hreg.T.reshape(8, 128, RB * 128).transpose(1, 0, 2)
        )
        # keys-major region, partition-major for contiguous DMA
        haug = np.ascontiguousarray(
            hreg.reshape(RB, 128, H).transpose(1, 0, 2)
        )

        in_maps.append(
            {
                "hTr": hTr,
                "haug": haug,
                "Wr": Wr,
                "maskd": mask,
                "bmaskd": bmask_pad if half == 0 else mask,
                "onesd": ones,
            }
        )
    return in_maps


def _assemble(results):
    out = np.empty((B, T, H), np.float32)
    for core in range(N_CORES):
        b, half = core // 2, core % 2
        out[b, half * QPC : (half + 1) * QPC] = (
            results[core]["out"].reshape(QPC, H).astype(np.float32)
        )
    return out


def kernel(h, W, T_hist):
    h = np.asarray(h, dtype=np.float32)
    W = np.asarray(W, dtype=np.float32)
    assert int(T_hist) == T_HIST
    assert h.shape == (B, T, H) and W.shape == (H, H)

    from concourse.bass_utils import run_bass_kernel_spmd

    nc = _build()
    in_maps = _prepare_in_maps(h, W)
    res = run_bass_kernel_spmd(nc, in_maps, core_ids=list(range(N_CORES)))
    return _assemble(res.results)
